# revision 14
# baseline (speedup 1.0000x reference)
"""3-layer GCN (PyG GCNConv-style) on 8 Trainium2 NeuronCores.

Design (dense-stream gather + one-hot PE segment-sum):
- dst nodes LPT-bin-packed into 784 (core,tile) bins of <=128, nodes relabeled
  (core,tile,row)-major; all feature tables stored in relabeled order.
- Per core, edges grouped by (tile j, src-chunk c) with compile-time budgets
  Q[j,c] = max-over-cores count (SPMD program; per-core variation is data).
- 4 src-chunks of 25000 rows (dma_gather int16 index limit), one gather stream
  per chunk on its own SWDGE queue (4 queues), calls of 1024 idx (64-desc/engine
  packet ceiling), 256B rows (L2 native 128-wide bf16; L3 duplicated 64-wide).
- Segment-sum: per 128-slot stream column, a one-hot sel built on DVE
  (dloc==iota) maps slots -> dst rows; PE matmuls accumulate into PSUM z per
  tile; self-loops via identity matmul over the tile's own table rows.
- L1 messages host-expanded (pure gather of x*dinv rows) and streamed with
  plain contiguous HWDGE DMA - no device gather for layer 1.
- Backend: z*dinv -> PE transpose -> GEMM (bias via aug row / K=1 ones matmul)
  -> celu (DVE+ACT) -> next table block; AllGather (internal Shared DRAM)
  between layers.
"""
import numpy as np
import ml_dtypes

bf16 = ml_dtypes.bfloat16

N = 100000
NC = 8
NPC = N // NC             # 12500
P = 128
TILES = 98                # 97*128 + 84
LAST_ROWS = NPC - 97 * P  # 84
NCHUNK = 4
CHUNK = N // NCHUNK       # 25000
CALL = 1024
LAST_EXEC_NS = None
LAST_TRACE = None


def _host_prep(edge_index):
    import heapq
    src0 = edge_index[0].astype(np.int64)
    dst0 = edge_index[1].astype(np.int64)
    deg = np.bincount(dst0, minlength=N).astype(np.float32) + 1.0  # self loop
    dinv = (1.0 / np.sqrt(deg)).astype(np.float32)

    # --- LPT pack dsts into 784 (core,tile) bins, balance by degree ---
    caps = np.full(NC * TILES, P, np.int64)
    caps[TILES - 1::TILES] = LAST_ROWS
    order = np.argsort(-deg, kind="stable")
    heap = [(0.0, b) for b in range(NC * TILES)]
    heapq.heapify(heap)
    members = [[] for _ in range(NC * TILES)]
    for v in order:
        while True:
            load, b = heapq.heappop(heap)
            if len(members[b]) < caps[b]:
                break
        members[b].append(int(v))
        if len(members[b]) < caps[b]:
            heapq.heappush(heap, (load + float(deg[v]), b))
    newid = np.empty(N, np.int64)
    for b in range(NC * TILES):
        c, j = b // TILES, b % TILES
        base = c * NPC + j * P
        mem = np.array(members[b], np.int64)
        newid[mem] = base + np.arange(len(mem))
    orig_of_new = np.empty(N, np.int64)
    orig_of_new[newid] = np.arange(N)

    sN = newid[src0]
    dN = newid[dst0]
    core = dN // NPC
    loc = dN % NPC
    j_of = loc // P                      # j=97 covers rows 12416..12499
    row_of = loc - j_of * P
    ch = sN // CHUNK

    cnt = np.bincount((core * TILES + j_of) * NCHUNK + ch,
                      minlength=NC * TILES * NCHUNK).reshape(NC, TILES, NCHUNK)
    Q = cnt.max(axis=0)                  # [TILES, NCHUNK] compile-time budgets
    cum = np.zeros((TILES, NCHUNK), np.int64)
    for c4 in range(NCHUNK):
        cum[:, c4] = np.concatenate([[0], np.cumsum(Q[:-1, c4])])
    S = Q.sum(axis=0)
    CALLS = [int(-(-S[c4] // CALL)) for c4 in range(NCHUNK)]
    Spad = [CALLS[c4] * CALL for c4 in range(NCHUNK)]

    entries = []                         # (j, c4, K) emission order
    entkey = {}
    for j in range(TILES):
        for c4 in range(NCHUNK):
            a, b2 = int(cum[j, c4]), int(cum[j, c4] + Q[j, c4])
            if b2 == a:
                continue
            for K in range(a // P, -(-b2 // P)):
                entkey[(j, c4, K)] = len(entries)
                entries.append((j, c4, K))
    NENT = len(entries)

    per_core = []
    for c8 in range(NC):
        m = core == c8
        ej, ec, erow, esrc = j_of[m], ch[m], row_of[m], sN[m]
        o = np.lexsort((erow, ec, ej))
        ej, ec, erow, esrc = ej[o], ec[o], erow[o], esrc[o]
        gkey = ej * NCHUNK + ec
        counts_g = np.bincount(gkey, minlength=TILES * NCHUNK)
        starts = np.concatenate([[0], np.cumsum(counts_g)[:-1]])
        rank = np.arange(len(gkey)) - np.repeat(starts, counts_g)
        pos = cum[ej, ec] + rank
        idx_streams = [np.zeros(Spad[c4], np.int16) for c4 in range(NCHUNK)]
        srcg_streams = [np.zeros(Spad[c4], np.int64) for c4 in range(NCHUNK)]
        dloc = np.full((P, NENT), -1.0, np.float32)
        for c4 in range(NCHUNK):
            mm = ec == c4
            idx_streams[c4][pos[mm]] = (esrc[mm] - c4 * CHUNK).astype(np.int16)
            srcg_streams[c4][pos[mm]] = esrc[mm]
            E = np.array([entkey[(int(jj), c4, int(pp) // P)]
                          for jj, pp in zip(ej[mm], pos[mm])], np.int64)
            dloc[pos[mm] % P, E] = erow[mm]
        dv_loc = dinv[orig_of_new[c8 * NPC:(c8 + 1) * NPC]]
        dinv_c = np.zeros((P, TILES), np.float32)
        for j in range(TILES):
            rows = P if j < TILES - 1 else LAST_ROWS
            dinv_c[:rows, j] = dv_loc[j * P:j * P + rows]
        per_core.append(dict(idx=idx_streams, srcg=srcg_streams, dloc=dloc,
                             dinvc=dinv_c))
    sched = dict(Q=Q, cum=cum, S=S, CALLS=CALLS, Spad=Spad,
                 entries=entries, NENT=NENT)
    return newid, orig_of_new, dinv, per_core, sched


def _np_reference(x, edge_index, W1, b1, W2, b2, W3, b3):
    src = np.concatenate([edge_index[0].astype(np.int64), np.arange(N)])
    dst = np.concatenate([edge_index[1].astype(np.int64), np.arange(N)])
    deg = np.bincount(dst, minlength=N).astype(np.float32)
    dinv = 1.0 / np.sqrt(deg)

    def agg(v):
        vs = v * dinv[:, None]
        z = np.zeros_like(v)
        np.add.at(z, dst, vs[src])
        return z * dinv[:, None]

    celu = lambda v: np.maximum(v, 0) + np.exp(np.minimum(v, 0)) - 1.0
    h1 = celu(agg(x) @ W1 + b1)
    h2 = celu(agg(h1) @ W2 + b2)
    return celu(agg(h2 @ W3) + b3).astype(np.float32)
